# revision 27
# baseline (speedup 1.0000x reference)
"""Distributed sparse-MoE routing kernel for 8 Trainium2 NeuronCores.

Math (reference): per token t (T = B*S = 32768, H = 1024, E = 8):
  logits = x @ W_router + b_router            [T, E]
  best   = argmax(logits)                     [T]
  best_p = softmax(logits).max(-1)            [T]
  order  = stable argsort(best)               [T]
  out[t] = (x[order[t]] @ W_expert + b_expert) * best_p[t]

Strategy (8 cores, c = 0..7 owns tokens [c*4096, (c+1)*4096)):
  - x replicated to every core (bf16) + a per-core fp16-split transposed
    shard for the exact router matmul (PE fp32 matmul is a low-precision
    emulation that flips argmax on near ties; 4 exact fp16 passes give
    ~3e-7 on the logits).
  Phase 1 (local, DMA-bound): stream the fp16-split shard in 2MB
    partition-contiguous batches (first two at the head of the HWDGE FIFO);
    per 4-chunk batch: router matmuls into one PSUM tile, batched DVE
    post-processing (logits assembly, argmax, one-hot, softmax max-prob).
    A dummy collective at t=0 absorbs the ~56us ncfw startup; a dummy
    dma_gather preloads the Q7 gather library.
  Phase 2: per-expert counts (one strided reduce) -> tiny AllGather fired
    immediately; local sort metadata computed under the collective; the
    scatter destinations are wrap-permuted within each owner slice
    ((i&15)*256+((i>>7)&31)*8+((i>>4)&7)) so the ReduceScatter output lands
    directly in the Q7 gather-index layout; indirect-DMA scatter of token
    ids; ReduceScatter(add) -> own slice of the global order.
  Phase 3 (local, PE-bound): one contiguous cast-load of the indices,
    dma_gather of bf16 x rows by order (X-bar delivers rows already in the
    [h%128, k, tok] stationary layout), bf16 expert GEMM (fp32 accum,
    back-to-back matmuls at the PE clock), scale by best_p, write own
    output shard. Host concatenates shards.
"""

import numpy as np

import concourse.bass as bass
import concourse.mybir as mybir
from concourse import bacc
from concourse.tile import TileContext
from concourse.masks import make_upper_triangular, make_identity

P = 128
H = 1024
E = 8
CORES = 8
KT = H // P  # 8 k-tiles of the contraction dim
CB = 4       # chunks per phase-1 batch

F32 = mybir.dt.float32
I32 = mybir.dt.int32
U32 = mybir.dt.uint32
ADD = mybir.AluOpType.add
SUB = mybir.AluOpType.subtract
MUL = mybir.AluOpType.mult
EQ = mybir.AluOpType.is_equal
AX = mybir.AxisListType.X


def build(tokens_per_core: int):
    M = tokens_per_core // P  # chunks per core
    T = CORES * tokens_per_core
    NB = M // CB              # phase-1 batches
    nc = bacc.Bacc("TRN2", num_devices=CORES)

    # xs: fp16 2-way split of the transposed shard, partition-contiguous:
    # xs[p, m, k, s, t] = split_s[k*128+p, m*128+t], s=0 -> fp16(x),
    # s=1 -> fp16((x - fp16(x)) * 2048).
    xs = nc.declare_dram_parameter(
        "xs", [P, M, KT, 2, P], mybir.dt.float16, isOutput=False
    )
    wcat = nc.declare_dram_parameter(
        "wcat", [H, 2 * E], mybir.dt.float16, isOutput=False
    )
    x_full = nc.declare_dram_parameter(
        "x_full", [T, H], mybir.dt.bfloat16, isOutput=False
    )
    w_eb = nc.declare_dram_parameter("w_eb", [H, H], mybir.dt.bfloat16, isOutput=False)
    b_r = nc.declare_dram_parameter("b_r", [1, E], F32, isOutput=False)
    b_e = nc.declare_dram_parameter("b_e", [1, H], F32, isOutput=False)
    tb = nc.declare_dram_parameter("tb", [1, 1], F32, isOutput=False)
    ohme = nc.declare_dram_parameter("ohme", [1, E], F32, isOutput=False)
    out = nc.declare_dram_parameter("out", [tokens_per_core, H], F32, isOutput=True)

    cnt_src = nc.dram_tensor("cnt_src", [1, E], F32)
    cnt_all = nc.dram_tensor("cnt_all", [1, CORES * E], F32, addr_space="Shared")
    order_buf = nc.dram_tensor("order_buf", [T, 1], F32)
    order_rs = nc.dram_tensor("order_rs", [tokens_per_core, 1], F32)
    wa_src = nc.dram_tensor("wa_src", [1, 1], F32)
    wa_all = nc.dram_tensor("wa_all", [1, CORES], F32, addr_space="Shared")

    rg = [list(range(CORES))]

    with TileContext(nc) as tc:
        with (
            tc.tile_pool(name="cst", bufs=1) as cst,
            tc.tile_pool(name="meta", bufs=1) as meta,
            tc.tile_pool(name="sb", bufs=3) as sb,
            tc.tile_pool(name="sbr", bufs=3) as sbr,
        ):
            # warmup collective first: pays the ncfw mesh-init during phase 1
            nc.gpsimd.collective_compute(
                "AllGather", mybir.AluOpType.bypass, replica_groups=rg,
                ins=[wa_src[:, :]], outs=[wa_all[:, :]],
            )
            # head of the sync HWDGE FIFO: first xs batches + router weights,
            # so phase-1 compute starts ~6us in instead of ~22us
            xt_pre = []
            for b0 in range(min(2, NB)):
                xt = sbr.tile([P, CB, KT, 2, P], mybir.dt.float16, tag="xtm")
                nc.sync.dma_start(out=xt[:], in_=xs[:, b0 * CB : (b0 + 1) * CB])
                xt_pre.append(xt)
                if b0 == 0:
                    wc_t = cst.tile([P, KT, 2 * E], mybir.dt.float16, tag="wc")
                    nc.sync.dma_start(
                        out=wc_t[:],
                        in_=wcat[:, :].rearrange("(k p) e -> p k e", p=P),
                    )

            # ---------------- constants (overlap with the stream) ----------
            zbig = cst.tile([P, T // P], F32, tag="zbig")
            nc.vector.memset(zbig[:], 0.0)
            # zero the scatter target early (off the sync FIFO -> gpsimd)
            nc.gpsimd.dma_start(
                out=order_buf[:, :].rearrange("(p c) o -> p (c o)", p=P),
                in_=zbig[:],
            )
            tri = cst.tile([P, P], F32, tag="tri")
            make_upper_triangular(nc, tri[:], 1.0, diag=False)  # tri[s,t]=1 iff s<t
            ones = cst.tile([P, 1], F32, tag="ones")
            nc.vector.memset(ones[:], 1.0)
            zrow = cst.tile([1, 256], F32, tag="zrow")
            nc.vector.memset(zrow[:], 0.0)

            iota_e_i = cst.tile([P, E], I32, tag="iei")
            nc.gpsimd.iota(iota_e_i[:], pattern=[[1, E]], base=0, channel_multiplier=0)
            iota_e = cst.tile([P, E], F32, tag="ief")
            nc.vector.tensor_copy(out=iota_e[:], in_=iota_e_i[:])

            iota_t_i = cst.tile([P, M], I32, tag="iti")
            nc.gpsimd.iota(iota_t_i[:], pattern=[[P, M]], base=0, channel_multiplier=1)
            iota_t = cst.tile([P, M], F32, tag="itf")
            nc.vector.tensor_copy(out=iota_t[:], in_=iota_t_i[:])

            # preload the Q7 DMAGatherAnt library (~9us IRAM load) off the
            # critical path with a dummy gather
            idx0 = cst.tile([P, 8], mybir.dt.int16, tag="idx0")
            nc.vector.memset(idx0[:], 0)
            xtr0 = cst.tile([P, KT, P], mybir.dt.bfloat16, tag="xtr0")
            nc.gpsimd.dma_gather(
                xtr0[:], x_full[:, :], idx0[:], P, P, H, transpose=True
            )
            br_row = cst.tile([1, E], F32, tag="brr")
            nc.sync.dma_start(out=br_row[:], in_=b_r[:, :])
            br_b = cst.tile([P, E], F32, tag="brb")
            nc.gpsimd.partition_broadcast(br_b[:], br_row[:])
            be_row = cst.tile([1, H], F32, tag="ber")
            nc.sync.dma_start(out=be_row[:], in_=b_e[:, :])
            be_b = cst.tile([P, H], F32, tag="beb")
            nc.gpsimd.partition_broadcast(be_b[:], be_row[:])
            tb_row = cst.tile([1, 1], F32, tag="tbr")
            nc.sync.dma_start(out=tb_row[:], in_=tb[:, :])
            t0b = cst.tile([P, 1], F32, tag="t0b")
            nc.gpsimd.partition_broadcast(t0b[:], tb_row[:])
            ohme_row = cst.tile([1, E], F32, tag="ohmer")
            nc.sync.dma_start(out=ohme_row[:], in_=ohme[:, :])

            # persistent per-token metadata
            oh_all = meta.tile([P, M * E], F32, tag="ohall")
            bp_all = meta.tile([P, M], F32, tag="bpall")
            ssum_all = meta.tile([P, M], F32, tag="ssumall")
            mxi_all = meta.tile([P, M, E], U32, tag="mxiall")
            mx_all = meta.tile([P, M, E], F32, tag="mxall")
            best_all = meta.tile([P, M], F32, tag="bestall")
            negm_all = meta.tile([P, M], F32, tag="negmall")

            # ---------------- phase 1: router ----------------
            with (
                tc.tile_pool(name="psr", bufs=2, space="PSUM") as psr,
                tc.tile_pool(name="sbx", bufs=2) as sbx,
            ):
                for b0 in range(NB):
                    m0 = b0 * CB
                    if b0 < len(xt_pre):
                        xt4 = xt_pre[b0]
                    else:
                        xt4 = sbr.tile([P, CB, KT, 2, P], mybir.dt.float16, tag="xtm")
                        nc.sync.dma_start(out=xt4[:], in_=xs[:, m0 : m0 + CB])
                    # router matmuls: one PSUM tile per batch, slice (ci, j)
                    lps = psr.tile([P, CB, 2, 2 * E], F32, tag="lps")
                    for ci in range(CB):
                        for j in range(2):
                            for k in range(KT):
                                nc.tensor.matmul(
                                    lps[:, ci, j, :],
                                    lhsT=xt4[:, ci, k, j, :],
                                    rhs=wc_t[:, k, :],
                                    start=(k == 0),
                                    stop=(k == KT - 1),
                                    skip_group_check=True,
                                )
                    # logits = x1w1 + (x1w2' + x2'w1)/S + x2'w2'/S^2, S=2048
                    lg = sbx.tile([P, CB, E], F32, tag="lg")
                    nc.vector.tensor_tensor(
                        out=lg[:],
                        in0=lps[:, :, 0, 0:E],
                        in1=br_b[:, None, :].to_broadcast([P, CB, E]),
                        op=ADD,
                    )
                    for src, sc in (
                        (lps[:, :, 0, E : 2 * E], 1.0 / 2048.0),
                        (lps[:, :, 1, 0:E], 1.0 / 2048.0),
                        (lps[:, :, 1, E : 2 * E], 1.0 / (2048.0 * 2048.0)),
                    ):
                        nc.vector.scalar_tensor_tensor(
                            out=lg[:], in0=src, scalar=sc, in1=lg[:],
                            op0=MUL, op1=ADD,
                        )
                    for ci in range(CB):
                        m = m0 + ci
                        nc.vector.max(mx_all[:, m, :], lg[:, ci, :])
                        nc.vector.max_index(mxi_all[:, m, :], mx_all[:, m, :], lg[:, ci, :])
                    # batched: best (cast), one-hot, -max
                    nc.vector.tensor_copy(
                        out=best_all[:, m0 : m0 + CB], in_=mxi_all[:, m0 : m0 + CB, 0]
                    )
                    nc.vector.tensor_tensor(
                        out=oh_all[:, m0 * E : (m0 + CB) * E].rearrange(
                            "p (m e) -> p m e", e=E
                        ),
                        in0=iota_e[:, None, :].to_broadcast([P, CB, E]),
                        in1=best_all[:, m0 : m0 + CB, None].to_broadcast([P, CB, E]),
                        op=EQ,
                    )
                    nc.vector.tensor_scalar_mul(
                        negm_all[:, m0 : m0 + CB], mx_all[:, m0 : m0 + CB, 0], -1.0
                    )
                    for ci in range(CB):
                        m = m0 + ci
                        ex = sbx.tile([P, E], F32, tag="ex")
                        nc.scalar.activation(
                            ex[:],
                            lg[:, ci, :],
                            mybir.ActivationFunctionType.Exp,
                            bias=negm_all[:, m : m + 1],
                            scale=1.0,
                            accum_out=ssum_all[:, m : m + 1],
                        )

                # expert weights: issued after the xs batches on the same
                # HWDGE FIFO so they don't steal phase-1 stream bandwidth
                we_b = cst.tile([P, KT, H], mybir.dt.bfloat16, tag="web")
                nc.scalar.dma_start(
                    out=we_b[:], in_=w_eb[:, :].rearrange("(k p) n -> p k n", p=P)
                )

                # ---------------- counts -> AllGather ASAP ----------------
                with tc.tile_pool(name="psc", bufs=1, space="PSUM") as psc:
                    cc_ps = psc.tile([1, M * E], F32, tag="ccps")
                    nc.tensor.matmul(
                        cc_ps[:], lhsT=ones[:], rhs=oh_all[:], start=True, stop=True
                    )
                    ccrow = meta.tile([1, M * E], F32, tag="ccrow")
                    nc.vector.tensor_copy(out=ccrow[:], in_=cc_ps[:])
                tot_row = meta.tile([1, E], F32, tag="totrow")
                nc.vector.tensor_reduce(
                    out=tot_row[:],
                    in_=ccrow[:, :].rearrange("o (m e) -> o e m", e=E),
                    axis=AX, op=ADD,
                )
                nc.sync.dma_start(out=cnt_src[:, :], in_=tot_row[:])
                nc.gpsimd.collective_compute(
                    "AllGather", mybir.AluOpType.bypass, replica_groups=rg,
                    ins=[cnt_src[:, :]], outs=[cnt_all[:, :]],
                )

                # ---------- local sort metadata (runs under the AllGather) ----
                binc = meta.tile([1, M * E], F32, tag="binc")
                for e in range(E):
                    cc_e = ccrow[:, :].rearrange("o (m e) -> o e m", e=E)[:, e, :]
                    bi_e = binc[:, :].rearrange("o (m e) -> o e m", e=E)[:, e, :]
                    nc.vector.tensor_tensor_scan(
                        out=bi_e, data0=zrow[:, :M], data1=cc_e,
                        initial=0.0, op0=ADD, op1=ADD,
                    )
                bexc = meta.tile([1, M * E], F32, tag="bexc")
                nc.vector.tensor_tensor(out=bexc[:], in0=binc[:], in1=ccrow[:], op=SUB)
                with tc.tile_pool(name="psp", bufs=1, space="PSUM") as psp:
                    pexc_ps = psp.tile([P, M * E], F32, tag="pexcps")
                    nc.tensor.matmul(
                        pexc_ps[:], lhsT=tri[:], rhs=oh_all[:], start=True, stop=True
                    )
                    pexc = meta.tile([P, M * E], F32, tag="pexc")
                    nc.vector.tensor_copy(out=pexc[:], in_=pexc_ps[:])
                bexc_b = meta.tile([P, M * E], F32, tag="bexcb")
                nc.gpsimd.partition_broadcast(bexc_b[:], bexc[:])
                t256 = meta.tile([P, M * E], F32, tag="t256")
                nc.vector.tensor_tensor(out=t256[:], in0=pexc[:], in1=bexc_b[:], op=ADD)
                nc.vector.tensor_tensor(out=t256[:], in0=t256[:], in1=oh_all[:], op=MUL)
                dloc = meta.tile([P, M], F32, tag="dloc")
                nc.vector.tensor_reduce(
                    out=dloc[:], in_=t256[:, :].rearrange("p (m e) -> p m e", e=E),
                    axis=AX, op=ADD,
                )
                sglob = meta.tile([P, M], F32, tag="sglob")
                nc.vector.tensor_tensor(
                    out=sglob[:], in0=iota_t[:], in1=t0b[:].to_broadcast([P, M]), op=ADD
                )
                nc.vector.reciprocal(bp_all[:], ssum_all[:])

                # ---------------- phase 2: global rank + scatter + RS --------
                cntrow = meta.tile([1, CORES * E], F32, tag="cntrow")
                nc.sync.dma_start(out=cntrow[:], in_=cnt_all[:, :])
                # exclusive prefix over cores (per expert)
                cum = meta.tile([1, CORES * E], F32, tag="cum")
                nc.vector.tensor_copy(out=cum[:, 0:E], in_=zrow[:, 0:E])
                for c in range(1, CORES):
                    nc.vector.tensor_tensor(
                        out=cum[:, c * E : (c + 1) * E],
                        in0=cum[:, (c - 1) * E : c * E],
                        in1=cntrow[:, (c - 1) * E : c * E],
                        op=ADD,
                    )
                totE = meta.tile([1, E], F32, tag="totE")
                nc.vector.tensor_tensor(
                    out=totE[:],
                    in0=cum[:, (CORES - 1) * E :],
                    in1=cntrow[:, (CORES - 1) * E :],
                    op=ADD,
                )
                eb_inc = meta.tile([1, E], F32, tag="ebinc")
                nc.vector.tensor_tensor_scan(
                    out=eb_inc[:], data0=zrow[:, :E], data1=totE[:],
                    initial=0.0, op0=ADD, op1=ADD,
                )
                eb_exc = meta.tile([1, E], F32, tag="ebexc")
                nc.vector.tensor_tensor(out=eb_exc[:], in0=eb_inc[:], in1=totE[:], op=SUB)
                # select my core's column of cum: radd0[e] = cum[me, e]
                tsel = meta.tile([1, CORES * E], F32, tag="tsel")
                nc.vector.tensor_tensor(
                    out=tsel[:, :].rearrange("o (c e) -> o e c", e=E),
                    in0=cum[:, :].rearrange("o (c e) -> o e c", e=E),
                    in1=ohme_row[:, None, :].to_broadcast([1, E, CORES]),
                    op=MUL,
                )
                radd0 = meta.tile([1, E], F32, tag="radd0")
                nc.vector.tensor_reduce(
                    out=radd0[:],
                    in_=tsel[:, :].rearrange("o (c e) -> o e c", e=E),
                    axis=AX, op=ADD,
                )
                radd_row = meta.tile([1, E], F32, tag="raddrow")
                nc.vector.tensor_tensor(out=radd_row[:], in0=eb_exc[:], in1=radd0[:], op=ADD)
                radd_b = meta.tile([P, E], F32, tag="raddb")
                nc.gpsimd.partition_broadcast(radd_b[:], radd_row[:])

                t256b = meta.tile([P, M * E], F32, tag="t256b")
                nc.vector.tensor_tensor(
                    out=t256b[:, :].rearrange("p (m e) -> p m e", e=E),
                    in0=oh_all[:, :].rearrange("p (m e) -> p m e", e=E),
                    in1=radd_b[:, None, :].to_broadcast([P, M, E]),
                    op=MUL,
                )
                radd_sel = meta.tile([P, M], F32, tag="raddsel")
                nc.vector.tensor_reduce(
                    out=radd_sel[:], in_=t256b[:, :].rearrange("p (m e) -> p m e", e=E),
                    axis=AX, op=ADD,
                )
                rankg = meta.tile([P, M], F32, tag="rankg")
                nc.vector.tensor_tensor(out=rankg[:], in0=dloc[:], in1=radd_sel[:], op=ADD)
                rank_i = meta.tile([P, M], I32, tag="ranki")
                nc.vector.tensor_copy(out=rank_i[:], in_=rankg[:])
                # wrap-permute the scatter destination within the owner's
                # slice so the RS output lands directly in the Q7 dma_gather
                # index layout: i=rank&4095 -> (i&15)*256 + ((i>>7)&31)*8
                # + ((i>>4)&7), keeping the owner base (rank & ~4095).
                SHR = mybir.AluOpType.logical_shift_right
                SHL = mybir.AluOpType.logical_shift_left
                AND = mybir.AluOpType.bitwise_and
                q256 = meta.tile([P, M], I32, tag="q256")
                nc.vector.tensor_scalar(q256[:], rank_i[:], 15, 8, op0=AND, op1=SHL)
                s3 = meta.tile([P, M], I32, tag="s3")
                nc.vector.tensor_scalar(s3[:], rank_i[:], 4, 7, op0=SHR, op1=AND)
                m8 = meta.tile([P, M], I32, tag="m8")
                nc.vector.tensor_scalar(m8[:], rank_i[:], 4, 248, op0=SHR, op1=AND)
                fperm = meta.tile([P, M], I32, tag="fperm")
                nc.vector.tensor_scalar(fperm[:], rank_i[:], -4096, None, op0=AND)
                nc.vector.tensor_tensor(out=fperm[:], in0=fperm[:], in1=q256[:], op=ADD)
                nc.vector.tensor_tensor(out=fperm[:], in0=fperm[:], in1=m8[:], op=ADD)
                nc.vector.tensor_tensor(out=fperm[:], in0=fperm[:], in1=s3[:], op=ADD)

                # scatter own ids at global rank, then reduce-scatter
                SCB = 1  # chunks per indirect scatter call (multi-chunk offsets are not elementwise)
                with nc.semaphore("scat_sem") as ssem, tc.tile_critical():
                    for m in range(0, M, SCB):
                        nc.gpsimd.indirect_dma_start(
                            out=order_buf[:, :],
                            out_offset=bass.IndirectOffsetOnAxis(
                                ap=fperm[:, m : m + SCB], axis=0
                            ),
                            in_=sglob[:, m : m + SCB],
                            in_offset=None,
                        ).then_inc(ssem, 16)
                    nc.gpsimd.wait_ge(ssem, 16 * (M // SCB))
                nc.gpsimd.collective_compute(
                    "ReduceScatter", ADD, replica_groups=rg,
                    ins=[order_buf[:, :]], outs=[order_rs[:, :]],
                )

                # ---------------- phase 3: gather + expert GEMM ----------------
                # the scatter pre-permuted the ids, so the RS output is already
                # in the Q7 dma_gather index layout: one contiguous cast-load.
                idx_st = meta.tile([16, M * 8], mybir.dt.int16, tag="idxst")
                nc.gpsimd.dma_start(
                    out=idx_st[:],
                    in_=order_rs[:, :].rearrange("(q f) o -> q (f o)", q=16),
                )
                idx_rep = meta.tile([P, M * 8], mybir.dt.int16, tag="idxrep")
                for g in range(8):
                    eng = nc.sync if g % 2 == 0 else nc.scalar
                    eng.dma_start(
                        out=idx_rep[16 * g : 16 * (g + 1), :], in_=idx_st[:, :]
                    )

                with tc.tile_pool(name="psy", bufs=2, space="PSUM") as psy:
                    for m in range(M):
                        # gather 128 rows already transposed to [h%128, k, tok]
                        xtr = sb.tile([P, KT, P], mybir.dt.bfloat16, tag="xtr")
                        nc.gpsimd.dma_gather(
                            xtr[:],
                            x_full[:, :],
                            idx_rep[:, m * 8 : (m + 1) * 8],
                            P,
                            P,
                            H,
                            transpose=True,
                        )
                        yps = psy.tile([P, H], F32, tag="yps")
                        for k in range(KT):
                            for nh in range(2):
                                nc.tensor.matmul(
                                    yps[:, nh * 512 : (nh + 1) * 512],
                                    lhsT=xtr[:, k, :],
                                    rhs=we_b[:, k, nh * 512 : (nh + 1) * 512],
                                    start=(k == 0),
                                    stop=(k == KT - 1),
                                    skip_group_check=True,
                                )
                        yb = sb.tile([P, H], F32, tag="yb")
                        nc.vector.tensor_tensor(out=yb[:], in0=yps[:], in1=be_b[:], op=ADD)
                        ys = sb.tile([P, H], F32, tag="ys")
                        nc.scalar.mul(ys[:], yb[:], bp_all[:, m : m + 1])
                        nc.sync.dma_start(out=out[m * P : (m + 1) * P, :], in_=ys[:])

    nc.compile()
    return nc


_NC_CACHE = {}


def _get_nc(tokens_per_core: int):
    if tokens_per_core not in _NC_CACHE:
        _NC_CACHE[tokens_per_core] = build(tokens_per_core)
    return _NC_CACHE[tokens_per_core]


def make_in_maps(x, W_router, b_router, W_expert, b_expert):
    B, S, Hh = x.shape
    assert Hh == H
    T = B * S
    tpc = T // CORES
    import ml_dtypes

    h = np.ascontiguousarray(x.reshape(T, H), dtype=np.float32)
    SPLIT = np.float32(2048.0)
    w_r = np.ascontiguousarray(W_router, dtype=np.float32)
    w1 = w_r.astype(np.float16)
    w2 = ((w_r - w1.astype(np.float32)) * SPLIT).astype(np.float16)
    wcat = np.ascontiguousarray(np.concatenate([w1, w2], axis=1))
    h_bf = h.astype(ml_dtypes.bfloat16)
    w_eb = np.ascontiguousarray(W_expert, dtype=np.float32).astype(ml_dtypes.bfloat16)
    maps = []
    for c in range(CORES):
        xT = np.ascontiguousarray(h[c * tpc : (c + 1) * tpc].T)  # [H, tpc] f32
        x1 = xT.astype(np.float16)
        x2 = ((xT - x1.astype(np.float32)) * SPLIT).astype(np.float16)
        M = tpc // P
        # [p, m, k, s, t]: split_s[k*128+p, m*128+t]
        x1r = x1.reshape(KT, P, M, P).transpose(1, 2, 0, 3)
        x2r = x2.reshape(KT, P, M, P).transpose(1, 2, 0, 3)
        xs = np.ascontiguousarray(np.stack([x1r, x2r], axis=3))
        ohme = np.zeros((1, E), np.float32)
        ohme[0, c] = 1.0
        maps.append({
            "xs": xs,
            "wcat": wcat,
            "x_full": h_bf,
            "w_eb": w_eb,
            "b_r": np.ascontiguousarray(b_router, dtype=np.float32).reshape(1, E),
            "b_e": np.ascontiguousarray(b_expert, dtype=np.float32).reshape(1, H),
            "tb": np.array([[c * tpc]], np.float32),
            "ohme": ohme,
        })
    return maps


def kernel(x, W_router, b_router, W_expert, b_expert, _trace=False):
    x = np.asarray(x)
    B, S, _ = x.shape
    tpc = B * S // CORES
    nc = _get_nc(tpc)
    maps = make_in_maps(x, W_router, b_router, W_expert, b_expert)
    from concourse.bass_utils import run_bass_kernel_spmd

    if _trace:
        # register the NTFF profile hook that the agent image lacks
        try:
            import sys as _sys, types as _types
            import trn_agent_boot.trn_boot as _tb
            _hook = _tb._ntff_profile_via_ctypes("/opt/axon/libaxon_pjrt.so")
            _mod = _types.ModuleType("antenv.axon_hooks")
            _mod.get_axon_ntff_profile_hook = lambda: _hook
            _sys.modules["antenv.axon_hooks"] = _mod
        except Exception:
            _trace = False
    try:
        res = run_bass_kernel_spmd(
            nc, maps, core_ids=list(range(CORES)), trace=_trace,
        )
    except Exception:
        if not _trace:
            raise
        res = run_bass_kernel_spmd(
            nc, maps, core_ids=list(range(CORES)), trace=False,
        )
    shards = [np.asarray(res.results[c]["out"]) for c in range(CORES)]
    full = np.concatenate(shards, axis=0).reshape(B, S, H)
    kernel.last_exec_time_ns = res.exec_time_ns
    return full


kernel.last_exec_time_ns = None
